# revision 1
# baseline (speedup 1.0000x reference)
"""Trainium2 Bass kernel for nn_Concentration_61229053772314.

kernel(**inputs) takes the FULL inputs (B=64), shards the batch dim across
8 NeuronCores (pure data parallel, weights replicated), runs a Bass/Tile
kernel via run_bass_kernel_spmd, and reassembles the full outputs.

v2 architecture (per core: NBA=256 (b,a) rows, 4 groups of GRP=64):
 - ve loaded once as f32, [n mod 128, (ba, u, h)] layout, 1MB DMAs.
 - compat = (ve * bcast(t)) summed over h: PE f32r outer-product
   broadcasts t (and topk indices) across partitions via ones^T @ row;
   gpsimd does the elementwise multiply (in-place on venat); DVE does a
   grouped tensor_reduce over h.  No per-ba ops, no bf16 casts.
 - softmax/top-16 on [64,256] per group (DVE max8/max_index/match_replace).
 - one-hot selectors built directly in [n, (ba,j)] orientation by
   comparing a broadcast index matrix against the partition index.
 - stage-3: per-ba fp32 matmuls ve^T @ [onehot|score] -> gathered rows +
   attention numerator u; W_fwd/W_mot heads as in reference.
"""
import math
import os
import sys

for _p in ("/opt/trn_rl_repo", "/root/.axon_site/_ro/trn_rl_repo"):
    if os.path.isdir(_p) and _p not in sys.path:
        sys.path.insert(0, _p)

import numpy as np
import concourse.tile as tile
from concourse import bacc, bass_utils, mybir

F32 = mybir.dt.float32
F32R = mybir.dt.float32r
I32 = mybir.dt.int32
U16 = mybir.dt.uint16
AX = mybir.AxisListType
ALU = mybir.AluOpType
ACTF = mybir.ActivationFunctionType

N_CORES = 8
B, A = 64, 32
N = 256    # entries per (b,a)
H = 128    # head dim
K16 = 16   # top-k
GRP = 64   # (b,a) pairs per processing group
QB = 8     # ba per DMA / pipeline step
NQ = GRP // QB  # QB steps per group

NEG_MASK = -1.0e30   # added to masked entries
NEG_REPL = -3.0e38   # match_replace fill (below any real/masked value)

_CACHE = {}


def _build(nc, B_pc):
    NBA = 32 * B_pc
    assert NBA % GRP == 0
    NG = NBA // GRP

    ve_d = nc.dram_tensor("ve", [NBA, N, H], F32, kind="ExternalInput")
    vs_d = nc.dram_tensor("vs", [NBA, H], F32, kind="ExternalInput")
    dead_d = nc.dram_tensor("dead", [NBA, N], I32, kind="ExternalInput")
    wq_d = nc.dram_tensor("wq", [H, H], F32, kind="ExternalInput")
    wk_d = nc.dram_tensor("wk", [H, H], F32, kind="ExternalInput")
    wv_d = nc.dram_tensor("wv", [H, H], F32, kind="ExternalInput")
    wmot_d = nc.dram_tensor("wmot", [H, 2 * H], F32, kind="ExternalInput")
    bmot_d = nc.dram_tensor("bmot", [H, 1], F32, kind="ExternalInput")
    wfwd_d = nc.dram_tensor("wfwd", [H, (K16 + 1) * H], F32, kind="ExternalInput")
    bfwd_d = nc.dram_tensor("bfwd", [H, 1], F32, kind="ExternalInput")
    vc_d = nc.dram_tensor("vc", [NBA, H], F32, kind="ExternalOutput")
    vm_d = nc.dram_tensor("vm", [NBA, H], F32, kind="ExternalOutput")

    with tile.TileContext(nc) as tc:
        _body(nc, tc, NBA, NG, ve_d, vs_d, dead_d, wq_d, wk_d, wv_d,
              wmot_d, bmot_d, wfwd_d, bfwd_d, vc_d, vm_d)


def _body(nc, tc, NBA, NG, ve_d, vs_d, dead_d, wq_d, wk_d, wv_d,
          wmot_d, bmot_d, wfwd_d, bfwd_d, vc_d, vm_d):
    from contextlib import ExitStack
    with ExitStack() as ctx:
        consts = ctx.enter_context(tc.tile_pool(name="consts", bufs=1))
        wres = ctx.enter_context(tc.tile_pool(name="wres", bufs=1))
        venat_pool = ctx.enter_context(tc.tile_pool(name="venat", bufs=10))
        tpk_pool = ctx.enter_context(tc.tile_pool(name="tpk", bufs=2))
        tbs_pool = ctx.enter_context(tc.tile_pool(name="tbs", bufs=2))
        mscr_pool = ctx.enter_context(tc.tile_pool(name="mscr", bufs=2))
        tpre = ctx.enter_context(tc.tile_pool(name="tpre", bufs=1))
        vst_pool = ctx.enter_context(tc.tile_pool(name="vst", bufs=4))
        grp_pool = ctx.enter_context(tc.tile_pool(name="grp", bufs=2))
        small = ctx.enter_context(tc.tile_pool(name="small", bufs=3))
        dram_pool = ctx.enter_context(tc.tile_pool(name="dram", bufs=4, space="DRAM"))
        # PSUM budget (8 banks of 2KB):
        #   ps_tbs "tbs" [128,1024]f32 (2 banks) x2 bufs      = 4 banks
        #   ps_xsel "xsel" [128,272]f32 x2 bufs               = 2 banks
        #   ps_tr "tr" [128,256]f32 x2 bufs                   = 2 banks
        ps_tbs = ctx.enter_context(tc.tile_pool(name="ps_tbs", bufs=2, space="PSUM"))
        ps_xsel = ctx.enter_context(tc.tile_pool(name="ps_xsel", bufs=2, space="PSUM"))
        ps_tr = ctx.enter_context(tc.tile_pool(name="ps_tr", bufs=2, space="PSUM"))

        # ---- constants ----
        iota_n = consts.tile([128, N], I32)
        nc.gpsimd.iota(iota_n[:], pattern=[[1, N]], base=0, channel_multiplier=0)
        iota_p = consts.tile([128, 1], F32)
        nc.gpsimd.iota(iota_p[:], pattern=[[0, 1]], base=0, channel_multiplier=1,
                       allow_small_or_imprecise_dtypes=True)
        iota_p2 = consts.tile([128, 1], F32)  # p + 128
        nc.gpsimd.iota(iota_p2[:], pattern=[[0, 1]], base=128, channel_multiplier=1,
                       allow_small_or_imprecise_dtypes=True)
        ident_f = consts.tile([128, 128], F32)
        nc.vector.tensor_scalar(ident_f[:], iota_n[:, 0:128], iota_p[:], None,
                                op0=ALU.is_equal)
        ones_f = consts.tile([1, 128], F32)
        nc.gpsimd.memset(ones_f[:], 1.0)
        ones_r = consts.tile([1, 128], F32R)
        nc.scalar.copy(ones_r[:], ones_f[:])

        def pe_transpose(dst_sb, src_sb, eng=nc.scalar):
            """dst[f, p] = src[p, f] via PE; dst in SBUF (any engine drains)."""
            p_in, f_in = src_sb.shape[0], src_sb.shape[1]
            ps = ps_tr.tile([128, 256], F32, tag="tr")
            out = ps[0:f_in, 0:p_in]
            nc.tensor.transpose(out, src_sb, ident_f[0:p_in, 0:p_in])
            eng.copy(dst_sb, out)

        # ---- weights (temp pool closed after preamble) ----
        with tc.tile_pool(name="wtmp", bufs=1) as wtmp:
            wq = wres.tile([H, H], F32)
            nc.sync.dma_start(wq[:], wq_d.ap())
            wk = wtmp.tile([H, H], F32)
            nc.sync.dma_start(wk[:], wk_d.ap())
            wv = wtmp.tile([H, H], F32)
            nc.sync.dma_start(wv[:], wv_d.ap())
            wmot = wtmp.tile([H, 2 * H], F32)
            nc.sync.dma_start(wmot[:], wmot_d.ap())
            wfwd = wtmp.tile([H, (K16 + 1) * H], F32)
            nc.sync.dma_start(wfwd[:], wfwd_d.ap())
            bmot = wres.tile([H, 1], F32)
            nc.sync.dma_start(bmot[:], bmot_d.ap())
            bfwd = wres.tile([H, 1], F32)
            nc.sync.dma_start(bfwd[:], bfwd_d.ap())

            wkT = wres.tile([H, H], F32)
            pe_transpose(wkT[:], wk[:])
            wvT = wtmp.tile([H, H], F32)
            pe_transpose(wvT[:], wv[:])
            wm0T = wres.tile([H, H], F32)
            pe_transpose(wm0T[:], wmot[:, 0:H])
            wm1T = wtmp.tile([H, H], F32)
            pe_transpose(wm1T[:], wmot[:, H:2 * H])

            # WmvT = (Wm1 @ Wv^T)^T
            wmvT_f = wres.tile([H, H], F32)
            ps = ps_tr.tile([128, 256], F32, tag="tr")
            nc.tensor.matmul(ps[:, 0:128], wvT[:], wm1T[:])
            nc.scalar.copy(wmvT_f[:], ps[:, 0:128])

            # W_fwd block transposes [h, ho] packed [128, 17*128] (f32)
            wfT = wres.tile([H, (K16 + 1) * H], F32)
            for j in range(K16 + 1):
                pe_transpose(wfT[:, j * H:(j + 1) * H], wfwd[:, j * H:(j + 1) * H])

        # ---- per-group t precompute ----
        vst_f, t_dram_g = [], []
        for g in range(NG):
            vs_rows = tpre.tile([GRP, H], F32, tag="vsrows")
            nc.sync.dma_start(vs_rows[:], vs_d.ap()[g * GRP:(g + 1) * GRP, :])
            vstf = vst_pool.tile([H, GRP], F32, tag="vstf")
            pe_transpose(vstf[:], vs_rows[:])
            qt = tpre.tile([H, GRP], F32, tag="qt")
            ps = ps_tr.tile([128, 256], F32, tag="tr")
            nc.tensor.matmul(ps[:, 0:GRP], wq[:], vstf[:])
            nc.scalar.copy(qt[:], ps[:, 0:GRP])
            tsb = tpre.tile([H, GRP], F32, tag="tsb")
            ps = ps_tr.tile([128, 256], F32, tag="tr")
            nc.tensor.matmul(ps[:, 0:GRP], wkT[:], qt[:])
            nc.scalar.mul(tsb[:], ps[:, 0:GRP], 1.0 / math.sqrt(H))
            # t rows [ba, h]: split into f32r hi + lo residual (exact in f32)
            trows_f = tpre.tile([GRP, H], F32, tag="trowsf")
            pe_transpose(trows_f[:], tsb[:])
            trows_r = tpre.tile([GRP, H], F32R, tag="trowsr")
            nc.scalar.copy(trows_r[:], trows_f[:])
            tlo_r = tpre.tile([GRP, H], F32R, tag="tlor")
            nc.vector.tensor_tensor(tlo_r[:], trows_f[:], trows_r[:].bitcast(F32),
                                    op=ALU.subtract)
            t_dram = dram_pool.tile([2 * GRP, H], F32R, tag="tdram")
            nc.sync.dma_start(t_dram[:][0:GRP, :], trows_r[:])
            nc.sync.dma_start(t_dram[:][GRP:2 * GRP, :], tlo_r[:])
            vst_f.append(vstf)
            t_dram_g.append(t_dram)

        # ---- pipelined groups ----
        venat_g = {}     # g -> list of venat tiles (len NQ)
        cc_g = {}        # g -> cc tile [128, 2*GRP] ([n-half, (u, ba)])
        sel_g = {}       # g -> (s_a, s_b)
        xq_g = {}        # g -> gathered tile [128, GRP*17]
        u_g = {}         # g -> u tile [128, GRP]

        def emit_stage1_qb(g, q):
            """load QB ba's of ve, broadcast t, multiply, reduce -> cc cols."""
            ib = g * GRP + q * QB
            if q == 0:
                cc_g[g] = grp_pool.tile([128, 2 * GRP], F32, tag="cc", name="cc")
                venat_g[g] = []
            venat = venat_pool.tile([128, QB * N], F32, tag="venat")
            src = ve_d.ap()[ib:ib + QB].rearrange("b (u n) h -> n b u h", u=2)
            eng = nc.sync if q % 2 == 0 else nc.scalar
            eng.dma_start(venat[:].rearrange("p (b u h) -> p b u h", b=QB, u=2), src)
            venat_g[g].append(venat)
            # t pack for these QB rows: hi + lo halves [1, QB*H] f32r each
            tpk = tpk_pool.tile([1, QB * H], F32R, tag="tpk")
            nc.sync.dma_start(
                tpk[:], t_dram_g[g][:][q * QB:(q + 1) * QB, :].rearrange("b h -> (b h)"))
            tpl = tpk_pool.tile([1, QB * H], F32R, tag="tpl")
            nc.sync.dma_start(
                tpl[:], t_dram_g[g][:][GRP + q * QB:GRP + (q + 1) * QB, :]
                .rearrange("b h -> (b h)"))
            # PE outer-product broadcast (hi + lo accumulated) -> PSUM -> SBUF
            tbs_ps = ps_tbs.tile([128, QB * H], F32, tag="tbs")
            nc.tensor.matmul(tbs_ps[:, 0:512], ones_r[:], tpk[:, 0:512],
                             start=True, stop=False)
            nc.tensor.matmul(tbs_ps[:, 0:512], ones_r[:], tpl[:, 0:512],
                             start=False, stop=True)
            nc.tensor.matmul(tbs_ps[:, 512:1024], ones_r[:], tpk[:, 512:1024],
                             start=True, stop=False)
            nc.tensor.matmul(tbs_ps[:, 512:1024], ones_r[:], tpl[:, 512:1024],
                             start=False, stop=True)
            tbs = tbs_pool.tile([128, QB * H], F32, tag="tbs")
            nc.scalar.copy(tbs[:], tbs_ps[:])
            # multiply ve * t into scratch (venat must stay pristine for
            # stage-3).  gpsimd TT is ~2.2x slower than DVE, so give gpsimd
            # ~2/3 of the halves and DVE ~1/3 (DVE also does all reduces).
            vfull = venat[:].rearrange("p (b u h) -> p b u h", b=QB, u=2)
            vv0 = vfull[:, :, 0, :]
            vv1 = vfull[:, :, 1, :]
            scr = mscr_pool.tile([128, QB * N], F32, tag="mscr")
            sfull = scr[:].rearrange("p (u b h) -> p u b h", b=QB, u=2)
            sc0 = sfull[:, 0, :, :]
            sc1 = sfull[:, 1, :, :]
            tb = tbs[:].rearrange("p (b h) -> p b h", b=QB)
            step = g * NQ + q
            for ui, vv, sc in ((0, vv0, sc0), (1, vv1, sc1)):
                if (2 * step + ui) % 3 == 0:
                    nc.vector.scalar_tensor_tensor(sc, vv, 1.0, tb,
                                                   op0=ALU.mult, op1=ALU.mult)
                else:
                    nc.gpsimd.tensor_tensor(sc, vv, tb, op=ALU.mult)
            # grouped reduce over h -> cc[:, u*GRP + q*QB + b]
            cc = cc_g[g]
            nc.vector.tensor_reduce(cc[:, q * QB:(q + 1) * QB], sc0, axis=AX.X, op=ALU.add)
            nc.vector.tensor_reduce(cc[:, GRP + q * QB:GRP + (q + 1) * QB], sc1,
                                    axis=AX.X, op=ALU.add)

        def emit_stage2(g):
            """softmax + top-16 + selector build for group g."""
            cc = cc_g[g]
            # transpose compat halves: [n0|n1, ba] -> [ba, n]
            cmp_ps = ps_tr.tile([128, 256], F32, tag="tr")
            nc.tensor.transpose(cmp_ps[0:GRP, 0:128], cc[:, 0:GRP], ident_f[:])
            nc.tensor.transpose(cmp_ps[0:GRP, 128:256], cc[:, GRP:2 * GRP], ident_f[:])

            dead_i = grp_pool.tile([GRP, N], I32, tag="deadi")
            nc.sync.dma_start(dead_i[:], dead_d.ap()[g * GRP:(g + 1) * GRP, :])
            dead_f = grp_pool.tile([GRP, N], F32, tag="deadf")
            nc.gpsimd.tensor_copy(dead_f[:], dead_i[:])
            cm_sb = grp_pool.tile([GRP, N], F32, tag="cmsb")
            nc.vector.scalar_tensor_tensor(cm_sb[:], dead_f[:], NEG_MASK,
                                           cmp_ps[0:GRP, :], op0=ALU.mult, op1=ALU.add)

            mx_neg = small.tile([GRP, 1], F32, tag="mxneg")
            nc.vector.tensor_reduce(mx_neg[:], cm_sb[:], axis=AX.X, op=ALU.max,
                                    negate=True)
            score_un = grp_pool.tile([GRP, N], F32, tag="scoreun")
            ssum = small.tile([GRP, 1], F32, tag="ssum")
            nc.scalar.activation(score_un[:], cm_sb[:], ACTF.Exp,
                                 bias=mx_neg[:], scale=1.0, accum_out=ssum[:])
            rs = small.tile([GRP, 1], F32, tag="rs")
            nc.vector.reciprocal(rs[:], ssum[:])
            score_f = grp_pool.tile([GRP, N], F32, tag="scoref")
            nc.vector.tensor_scalar_mul(score_f[:], score_un[:], rs[:])

            # top-16 (two rounds of max8 + find_index8)
            mx8a = small.tile([GRP, 8], F32, tag="mx8a")
            nc.vector.max(mx8a[:], cm_sb[:])
            idx16 = small.tile([GRP, K16], U16, tag="idx16")
            nc.vector.max_index(idx16[:, 0:8], mx8a[:], cm_sb[:])
            cm2 = grp_pool.tile([GRP, N], F32, tag="cm2")
            nc.vector.match_replace(cm2[:], mx8a[:], cm_sb[:], NEG_REPL)
            mx8b = small.tile([GRP, 8], F32, tag="mx8b")
            nc.vector.max(mx8b[:], cm2[:])
            nc.vector.max_index(idx16[:, 8:16], mx8b[:], cm2[:])
            idx_r = small.tile([GRP, K16], F32R, tag="idxr")
            nc.vector.tensor_copy(idx_r[:], idx16[:])
            # bounce idx rows to DRAM, reload flat on one partition
            idx_dram = dram_pool.tile([GRP, K16], F32R, tag="idxdram")
            nc.sync.dma_start(idx_dram[:], idx_r[:])
            idx_pack = tpk_pool.tile([1, GRP * K16], F32R, tag="idxpack")
            nc.sync.dma_start(idx_pack[:], idx_dram[:].rearrange("b k -> (b k)"))
            # broadcast indices to all partitions: [128, (ba, j)]
            idx_ps = ps_tbs.tile([128, GRP * K16], F32, tag="tbs")
            nc.tensor.matmul(idx_ps[:, 0:512], ones_r[:], idx_pack[:, 0:512],
                             start=True, stop=True)
            nc.tensor.matmul(idx_ps[:, 512:1024], ones_r[:], idx_pack[:, 512:1024],
                             start=True, stop=True)
            # drain broadcast indices to SBUF (gpsimd cannot read PSUM)
            idx_sb = tbs_pool.tile([128, GRP * K16], F32, tag="idxsb")
            nc.scalar.copy(idx_sb[:], idx_ps[:])
            # selectors: s[p, ba, j] = (idx[ba, j] == n(p)) ; col 17 = score
            s_a = grp_pool.tile([128, GRP * (K16 + 1)], F32, tag="sa")
            s_b = grp_pool.tile([128, GRP * (K16 + 1)], F32, tag="sb")
            s_a_v = s_a[:].rearrange("p (b j) -> p b j", j=K16 + 1)
            s_b_v = s_b[:].rearrange("p (b j) -> p b j", j=K16 + 1)
            idx_v = idx_sb[:].rearrange("p (b j) -> p b j", j=K16)
            nc.gpsimd.tensor_scalar(s_a_v[:, :, 0:K16], idx_v, iota_p[:], None,
                                    op0=ALU.is_equal)
            nc.gpsimd.tensor_scalar(s_b_v[:, :, 0:K16], idx_v, iota_p2[:], None,
                                    op0=ALU.is_equal)
            # score columns: transpose [ba, n] -> [n, ba]
            st_ps = ps_tr.tile([128, 256], F32, tag="tr")
            nc.tensor.transpose(st_ps[:, 0:GRP], score_f[:, 0:128],
                                ident_f[0:GRP, 0:GRP])
            nc.tensor.transpose(st_ps[:, GRP:2 * GRP], score_f[:, 128:256],
                                ident_f[0:GRP, 0:GRP])
            nc.scalar.copy(s_a_v[:, :, K16], st_ps[:, 0:GRP])
            nc.scalar.copy(s_b_v[:, :, K16], st_ps[:, GRP:2 * GRP])
            sel_g[g] = (s_a, s_b)
            xq_g[g] = grp_pool.tile([128, GRP * (K16 + 1)], F32, tag="xq", name="xq")
            u_g[g] = grp_pool.tile([128, GRP], F32, tag="uf", name="uf")

        CH = 16  # ba per PSUM drain chunk in stage 3
        xsel_cur = [None]

        def emit_stage3_chunk(g, q):
            """stage-3 for ba cols [q*QB, (q+1)*QB) of group g."""
            s_a, s_b = sel_g[g]
            xq, u_f = xq_g[g], u_g[g]
            for col in range(q * QB, (q + 1) * QB):
                pos = col % CH
                if pos == 0:
                    xsel_cur[0] = ps_xsel.tile([128, CH * (K16 + 1)], F32, tag="xsel", name="xsel")
                xsel_ps = xsel_cur[0]
                venat = venat_g[g][col // QB]
                base = (col % QB) * N
                lo, hi = pos * 17, pos * 17 + 17
                nc.tensor.matmul(xsel_ps[:, lo:hi], venat[:, base:base + 128],
                                 s_a[:, col * 17:(col + 1) * 17],
                                 start=True, stop=False)
                nc.tensor.matmul(xsel_ps[:, lo:hi], venat[:, base + 128:base + 256],
                                 s_b[:, col * 17:(col + 1) * 17],
                                 start=False, stop=True)
                if pos == CH - 1:
                    c0 = col + 1 - CH
                    nc.scalar.copy(xq[:, c0 * 17:(col + 1) * 17], xsel_ps[:])
                    xv = xsel_ps[:].rearrange("p (b j) -> p b j", j=K16 + 1)
                    nc.scalar.copy(u_f[:, c0:col + 1], xv[:, :, K16])

        def emit_heads(g):
            """vC / vM heads for group g (after its stage-3 drains)."""
            xq, u_f = xq_g[g], u_g[g]
            xq_v = xq[:].rearrange("p (b j) -> p b j", j=K16 + 1)
            vc_ps = ps_tr.tile([128, 256], F32, tag="tr")
            nc.tensor.matmul(vc_ps[:, 0:GRP], wfT[:, 0:H], vst_f[g][:],
                             start=True, stop=False)
            for j in range(1, K16 + 1):
                nc.tensor.matmul(vc_ps[:, 0:GRP], wfT[:, j * H:(j + 1) * H],
                                 xq_v[:, :, j - 1],
                                 start=False, stop=(j == K16))
            vc_sb = grp_pool.tile([128, GRP], F32, tag="vcsb")
            nc.scalar.activation(vc_sb[:], vc_ps[:, 0:GRP], ACTF.Relu,
                                 bias=bfwd[:], scale=1.0)
            vc_rows = grp_pool.tile([GRP, H], F32, tag="vcrows")
            pe_transpose(vc_rows[:], vc_sb[:])
            nc.sync.dma_start(vc_d.ap()[g * GRP:(g + 1) * GRP, :], vc_rows[:])

            vm_ps = ps_tr.tile([128, 256], F32, tag="tr")
            nc.tensor.matmul(vm_ps[:, 0:GRP], wm0T[:], vst_f[g][:],
                             start=True, stop=False)
            nc.tensor.matmul(vm_ps[:, 0:GRP], wmvT_f[:], u_f[:],
                             start=False, stop=True)
            vm_sb = grp_pool.tile([128, GRP], F32, tag="vmsb")
            nc.scalar.activation(vm_sb[:], vm_ps[:, 0:GRP], ACTF.Relu,
                                 bias=bmot[:], scale=1.0)
            vm_rows = grp_pool.tile([GRP, H], F32, tag="vmrows")
            pe_transpose(vm_rows[:], vm_sb[:])
            nc.sync.dma_start(vm_d.ap()[g * GRP:(g + 1) * GRP, :], vm_rows[:])

        # ---- software-pipelined emission ----
        for q in range(NQ):
            emit_stage1_qb(0, q)
        for g in range(NG):
            emit_stage2(g)
            for q in range(NQ):
                emit_stage3_chunk(g, q)
                if g + 1 < NG:
                    emit_stage1_qb(g + 1, q)
            emit_heads(g)
            del venat_g[g]


def _get_compiled(B_pc):
    key = B_pc
    if key not in _CACHE:
        nc = bacc.Bacc("TRN2", target_bir_lowering=False, debug=False,
                       num_devices=N_CORES)
        _build(nc, B_pc)
        nc.compile()
        _CACHE[key] = nc
    return _CACHE[key]


def kernel(vs, ve, ve_dead, Wq, Wk, Wv, W_mot, b_mot, W_fwd, b_fwd,
           trace=False, trace_kwargs=None):
    vs = np.asarray(vs, dtype=np.float32)
    ve = np.asarray(ve, dtype=np.float32)
    ve_dead = np.asarray(ve_dead, dtype=np.int32)
    Bq, Aq = vs.shape[0], vs.shape[1]
    assert (Bq, Aq) == (B, A), (Bq, Aq)
    B_pc = B // N_CORES
    NBA = B_pc * A

    nc = _get_compiled(B_pc)

    shared = {
        "wq": np.ascontiguousarray(Wq, dtype=np.float32),
        "wk": np.ascontiguousarray(Wk, dtype=np.float32),
        "wv": np.ascontiguousarray(Wv, dtype=np.float32),
        "wmot": np.ascontiguousarray(W_mot, dtype=np.float32),
        "bmot": np.ascontiguousarray(b_mot, dtype=np.float32).reshape(H, 1),
        "wfwd": np.ascontiguousarray(W_fwd, dtype=np.float32),
        "bfwd": np.ascontiguousarray(b_fwd, dtype=np.float32).reshape(H, 1),
    }
    in_maps = []
    for c in range(N_CORES):
        sl = slice(c * B_pc, (c + 1) * B_pc)
        in_maps.append({
            "ve": np.ascontiguousarray(ve[sl].reshape(NBA, N, H)),
            "vs": np.ascontiguousarray(vs[sl].reshape(NBA, H)),
            "dead": np.ascontiguousarray(ve_dead[sl].reshape(NBA, N)),
            **shared,
        })

    res = bass_utils.run_bass_kernel_spmd(
        nc, in_maps, core_ids=list(range(N_CORES)),
        trace=trace, **(trace_kwargs or {}))

    vc = np.empty((B, A, H), dtype=np.float32)
    vm = np.empty((B, A, H), dtype=np.float32)
    for c in range(N_CORES):
        sl = slice(c * B_pc, (c + 1) * B_pc)
        vc[sl] = res.results[c]["vc"].reshape(B_pc, A, H)
        vm[sl] = res.results[c]["vm"].reshape(B_pc, A, H)
    kernel.last_results = res
    return (vc, vm)



# revision 9
# speedup vs baseline: 2.0659x; 2.0659x over previous
"""Trainium2 Bass kernel for nn_Concentration_61229053772314.

kernel(**inputs) takes the FULL inputs (B=64), shards the batch dim across
8 NeuronCores (pure data parallel, weights replicated), runs a Bass/Tile
kernel via run_bass_kernel_spmd, and reassembles the full outputs.

v3 architecture (per core: NBA=256 (b,a) rows, 2 groups of GRP=128):
 - ve streamed once as f32 in [n mod 128, (b, u, h)] layout, 1MB DMAs;
   each tile is consumed by stage-1 (compat) + cast to a bf16 copy for
   stage-3, then freed (f32 venat is transient).
 - compat = sum_h ve*t: t broadcast via ONE K=2 PE matmul per 512 cols
   (hi/lo f32r planes stacked on 2 partitions -> exact f32 sum); the
   multiply+reduce is split between gpsimd (plain mult, u=0 half) and
   DVE tensor_tensor_reduce (fused mult+reduce, u=1 half).
 - softmax/top-16 on [128, 256] rows; one-hot selectors built on DVE in
   bf16 (indices <= 255 are exact in bf16).
 - stage-3 gather: selector is the 17-col STATIONARY (cheap LDWEIGHTS),
   ve_bf16 is the moving operand (1 cyc/row). 4 ba per PSUM tile via
   col tile_position; one PE transpose per chunk restores [h, (ba,j)].
   Gathered values / u are bf16 (tol 2e-2); compat/top-k stay f32.
 - heads: W_fwd blocks 1..16 and W_mot@WvT in bf16 against gathered x;
   the vs-terms stay exact f32.
"""
import math
import os
import sys

for _p in ("/opt/trn_rl_repo", "/root/.axon_site/_ro/trn_rl_repo"):
    if os.path.isdir(_p) and _p not in sys.path:
        sys.path.insert(0, _p)

import numpy as np
import concourse.tile as tile
from concourse import bacc, bass_utils, mybir

F32 = mybir.dt.float32
F32R = mybir.dt.float32r
BF16 = mybir.dt.bfloat16
I32 = mybir.dt.int32
U16 = mybir.dt.uint16
AX = mybir.AxisListType
ALU = mybir.AluOpType
ACTF = mybir.ActivationFunctionType

N_CORES = 8
B, A = 64, 32
N = 256    # entries per (b,a)
H = 128    # head dim
K16 = 16   # top-k
GRP = 128  # (b,a) pairs per processing group
QB = 8     # ba per DMA / pipeline step
NQ = GRP // QB  # steps per group
CH = 4     # ba per stage-3 psum chunk
NCH = GRP // CH

NEG_MASK = -1.0e30   # added to masked entries
NEG_REPL = -3.0e38   # match_replace fill (below any real/masked value)

_CACHE = {}


def _build(nc, B_pc):
    NBA = 32 * B_pc
    assert NBA % GRP == 0
    NG = NBA // GRP

    ve_d = nc.dram_tensor("ve", [NBA, N, H], F32, kind="ExternalInput")
    vs_d = nc.dram_tensor("vs", [NBA, H], F32, kind="ExternalInput")
    dead_d = nc.dram_tensor("dead", [NBA, N], I32, kind="ExternalInput")
    wq_d = nc.dram_tensor("wq", [H, H], F32, kind="ExternalInput")
    wk_d = nc.dram_tensor("wk", [H, H], F32, kind="ExternalInput")
    wv_d = nc.dram_tensor("wv", [H, H], F32, kind="ExternalInput")
    wmot_d = nc.dram_tensor("wmot", [H, 2 * H], F32, kind="ExternalInput")
    bmot_d = nc.dram_tensor("bmot", [H, 1], F32, kind="ExternalInput")
    wfwd_d = nc.dram_tensor("wfwd", [H, (K16 + 1) * H], F32, kind="ExternalInput")
    bfwd_d = nc.dram_tensor("bfwd", [H, 1], F32, kind="ExternalInput")
    vc_d = nc.dram_tensor("vc", [NBA, H], F32, kind="ExternalOutput")
    vm_d = nc.dram_tensor("vm", [NBA, H], F32, kind="ExternalOutput")

    with tile.TileContext(nc) as tc:
        _body(nc, tc, NBA, NG, ve_d, vs_d, dead_d, wq_d, wk_d, wv_d,
              wmot_d, bmot_d, wfwd_d, bfwd_d, vc_d, vm_d)


def _body(nc, tc, NBA, NG, ve_d, vs_d, dead_d, wq_d, wk_d, wv_d,
          wmot_d, bmot_d, wfwd_d, bfwd_d, vc_d, vm_d):
    from contextlib import ExitStack
    with ExitStack() as ctx:
        consts = ctx.enter_context(tc.tile_pool(name="consts", bufs=1))
        wres = ctx.enter_context(tc.tile_pool(name="wres", bufs=1))
        venat_pool = ctx.enter_context(tc.tile_pool(name="venat", bufs=3))
        vbf_pool = ctx.enter_context(tc.tile_pool(name="vbf", bufs=18))
        scr_pool = ctx.enter_context(tc.tile_pool(name="scr", bufs=2))
        tbs_pool = ctx.enter_context(tc.tile_pool(name="tbs", bufs=2))
        tpk_pool = ctx.enter_context(tc.tile_pool(name="tpk", bufs=2))
        tpre = ctx.enter_context(tc.tile_pool(name="tpre", bufs=1))
        vst_pool = ctx.enter_context(tc.tile_pool(name="vst", bufs=2))
        grp_pool = ctx.enter_context(tc.tile_pool(name="grp", bufs=2))
        grp1 = ctx.enter_context(tc.tile_pool(name="grp1", bufs=1))
        xsb_pool = ctx.enter_context(tc.tile_pool(name="xsb", bufs=3))
        small = ctx.enter_context(tc.tile_pool(name="small", bufs=3))
        dram_pool = ctx.enter_context(tc.tile_pool(name="dram", bufs=4, space="DRAM"))
        # PSUM budget, bank-granular (8 banks of 2KB/partition):
        #   ps_tbs [128,1024]f32 (2 banks) x2 bufs     = 4 banks
        #   ps_tr  [128,256]f32 x1                     = 1 bank
        #   ps_trb [128,256]bf16 x1                    = 1 bank
        #   ps_x   [128,256]f32 persistent (2 chunks)  = 1 bank
        #   ps_xt  [128,512]bf16 persistent (2 chunks) = 1 bank
        ps_tbs = ctx.enter_context(tc.tile_pool(name="ps_tbs", bufs=2, space="PSUM"))
        ps_tr = ctx.enter_context(tc.tile_pool(name="ps_tr", bufs=1, space="PSUM"))
        ps_trb = ctx.enter_context(tc.tile_pool(name="ps_trb", bufs=1, space="PSUM"))
        ps_x = ctx.enter_context(tc.tile_pool(name="ps_x", bufs=1, space="PSUM"))
        ps_xt = ctx.enter_context(tc.tile_pool(name="ps_xt", bufs=1, space="PSUM"))

        # ---- constants ----
        iota_n = consts.tile([128, 128], I32)
        nc.gpsimd.iota(iota_n[:], pattern=[[1, 128]], base=0, channel_multiplier=0)
        iota_p = consts.tile([128, 1], F32)
        nc.gpsimd.iota(iota_p[:], pattern=[[0, 1]], base=0, channel_multiplier=1,
                       allow_small_or_imprecise_dtypes=True)
        iota_p2 = consts.tile([128, 1], F32)  # p + 128
        nc.gpsimd.iota(iota_p2[:], pattern=[[0, 1]], base=128, channel_multiplier=1,
                       allow_small_or_imprecise_dtypes=True)
        iota_p_bf = consts.tile([128, 1], BF16)
        nc.vector.tensor_copy(iota_p_bf[:], iota_p[:])
        iota_p2_bf = consts.tile([128, 1], BF16)
        nc.vector.tensor_copy(iota_p2_bf[:], iota_p2[:])
        ident_f = consts.tile([128, 128], F32)
        nc.vector.tensor_scalar(ident_f[:], iota_n[:], iota_p[:], None,
                                op0=ALU.is_equal)
        ident_bf = consts.tile([128, 128], BF16)
        nc.vector.tensor_copy(ident_bf[:], ident_f[:])
        ones2_f = consts.tile([2, 128], F32)
        nc.gpsimd.memset(ones2_f[:], 1.0)
        ones2_r = consts.tile([2, 128], F32R)
        nc.scalar.copy(ones2_r[:], ones2_f[:])
        ones1_bf = consts.tile([1, 128], BF16)
        nc.gpsimd.memset(ones1_bf[:], 1.0)

        def pe_transpose(dst_sb, src_sb, eng=nc.scalar):
            """dst[f, p] = src[p, f] via PE; dst in SBUF (f32 path)."""
            p_in, f_in = src_sb.shape[0], src_sb.shape[1]
            ps = ps_tr.tile([128, 256], F32, tag="tr")
            out = ps[0:f_in, 0:p_in]
            nc.tensor.transpose(out, src_sb, ident_f[0:p_in, 0:p_in])
            eng.copy(dst_sb, out)

        # ---- weights (temp pool closed after preamble) ----
        with tc.tile_pool(name="wtmp", bufs=1) as wtmp:
            wq = wres.tile([H, H], F32)
            nc.sync.dma_start(wq[:], wq_d.ap())
            wk = wtmp.tile([H, H], F32)
            nc.sync.dma_start(wk[:], wk_d.ap())
            wv = wtmp.tile([H, H], F32)
            nc.sync.dma_start(wv[:], wv_d.ap())
            wmot = wtmp.tile([H, 2 * H], F32)
            nc.sync.dma_start(wmot[:], wmot_d.ap())
            wfwd = wtmp.tile([H, (K16 + 1) * H], F32)
            nc.sync.dma_start(wfwd[:], wfwd_d.ap())
            bmot = wres.tile([H, 1], F32)
            nc.sync.dma_start(bmot[:], bmot_d.ap())
            bfwd = wres.tile([H, 1], F32)
            nc.sync.dma_start(bfwd[:], bfwd_d.ap())

            wkT = wres.tile([H, H], F32)
            pe_transpose(wkT[:], wk[:])
            wvT = wtmp.tile([H, H], F32)
            pe_transpose(wvT[:], wv[:])
            wm0T = wres.tile([H, H], F32)
            pe_transpose(wm0T[:], wmot[:, 0:H])
            wm1T = wtmp.tile([H, H], F32)
            pe_transpose(wm1T[:], wmot[:, H:2 * H])

            # WmvT = (Wm1 @ Wv^T)^T, in bf16 (multiplies bf16 u)
            wmvT_f = wtmp.tile([H, H], F32)
            ps = ps_tr.tile([128, 256], F32, tag="tr")
            nc.tensor.matmul(ps[:, 0:128], wvT[:], wm1T[:])
            nc.scalar.copy(wmvT_f[:], ps[:, 0:128])
            wmv_bf = wres.tile([H, H], BF16)
            nc.scalar.copy(wmv_bf[:], wmvT_f[:])

            # W_fwd block transposes; block 0 stays f32 (vs term), blocks
            # 1..16 drain to bf16 (they multiply bf16 gathered rows)
            wf0T = wres.tile([H, H], F32)
            pe_transpose(wf0T[:], wfwd[:, 0:H])
            wf_bf = wres.tile([H, K16 * H], BF16)
            for j in range(1, K16 + 1):
                pe_transpose(wf_bf[:, (j - 1) * H:j * H],
                             wfwd[:, j * H:(j + 1) * H])

        # ---- per-group t precompute ----
        vst_f, t_dram_g = [], []
        for g in range(NG):
            vs_rows = tpre.tile([GRP, H], F32, tag="vsrows")
            nc.sync.dma_start(vs_rows[:], vs_d.ap()[g * GRP:(g + 1) * GRP, :])
            vstf = vst_pool.tile([H, GRP], F32, tag="vstf")
            pe_transpose(vstf[:], vs_rows[:])
            qt = tpre.tile([H, GRP], F32, tag="qt")
            ps = ps_tr.tile([128, 256], F32, tag="tr")
            nc.tensor.matmul(ps[:, 0:GRP], wq[:], vstf[:])
            nc.scalar.copy(qt[:], ps[:, 0:GRP])
            tsb = tpre.tile([H, GRP], F32, tag="tsb")
            ps = ps_tr.tile([128, 256], F32, tag="tr")
            nc.tensor.matmul(ps[:, 0:GRP], wkT[:], qt[:])
            nc.scalar.mul(tsb[:], ps[:, 0:GRP], 1.0 / math.sqrt(H))
            # t rows [ba, h]: split into f32r hi + lo residual (exact in f32)
            trows_f = tpre.tile([GRP, H], F32, tag="trowsf")
            pe_transpose(trows_f[:], tsb[:])
            trows_r = tpre.tile([GRP, H], F32R, tag="trowsr")
            nc.scalar.copy(trows_r[:], trows_f[:])
            tlo_r = tpre.tile([GRP, H], F32R, tag="tlor")
            nc.vector.tensor_tensor(tlo_r[:], trows_f[:], trows_r[:].bitcast(F32),
                                    op=ALU.subtract)
            # t_dram[u, ba, h]: u=0 hi plane, u=1 lo plane
            t_dram = dram_pool.tile([2, GRP, H], F32R, tag="tdram")
            nc.sync.dma_start(t_dram[:][0], trows_r[:])
            nc.sync.dma_start(t_dram[:][1], tlo_r[:])
            vst_f.append(vstf)
            t_dram_g.append(t_dram)

        # ---- per-group state ----
        xps_all = ps_x.tile([128, 256], F32, tag="x")
        xt_all = ps_xt.tile([128, 256], BF16, tag="xt")
        vbf_g = {}       # (g, t8) -> bf16 venat tile [128, QB*N]
        cc_g = {}        # g -> cc tile [128, 2*GRP] ([n-half, (u, ba)])
        sel_g = {}       # g -> (s_a, s_b) bf16 [128, GRP*17]
        xq_g = {}        # g -> gathered tile [128, NCH*128] bf16

        def emit_stage1_qb(g, q):
            """load QB ba's of ve, broadcast t, multiply+reduce, cast bf16."""
            ib = g * GRP + q * QB
            if q == 0:
                cc_g[g] = grp_pool.tile([128, 2 * GRP], F32, tag="cc", name="cc")
            cc = cc_g[g]
            venat = venat_pool.tile([128, QB * N], F32, tag="venat")
            src = ve_d.ap()[ib:ib + QB].rearrange("b (u n) h -> n b u h", u=2)
            nc.sync.dma_start(
                venat[:].rearrange("p (b u h) -> p b u h", b=QB, u=2), src)
            # t pack for these QB rows: [2, QB*H] f32r (hi plane, lo plane)
            tpk = tpk_pool.tile([2, QB * H], F32R, tag="tpk")
            nc.sync.dma_start(
                tpk[:].rearrange("p (b h) -> p b h", b=QB),
                t_dram_g[g][:][:, q * QB:(q + 1) * QB, :])
            # broadcast t across partitions: K=2 matmul sums hi+lo exactly
            tbs_ps = ps_tbs.tile([128, QB * H], F32, tag="tbs")
            nc.tensor.matmul(tbs_ps[:, 0:512], ones2_r[:], tpk[:, 0:512],
                             start=True, stop=True)
            nc.tensor.matmul(tbs_ps[:, 512:1024], ones2_r[:], tpk[:, 512:1024],
                             start=True, stop=True)
            tbs = tbs_pool.tile([128, QB * H], F32, tag="tbs")
            nc.scalar.copy(tbs[:], tbs_ps[:])
            vfull = venat[:].rearrange("p (b u h) -> p b u h", b=QB, u=2)
            tb = tbs[:].rearrange("p (b h) -> p b h", b=QB)
            # multiply: gpsimd takes b < GB (both u), DVE the rest; DVE does
            # both grouped reduces.  scr layout [p, u, b, h].
            GB = 5
            scr = scr_pool.tile([128, QB * N], F32, tag="scr")
            s4 = scr[:].rearrange("p (u b h) -> p u b h", b=QB, u=2)
            for u in range(2):
                nc.gpsimd.tensor_tensor(s4[:, u, 0:GB, :], vfull[:, 0:GB, u, :],
                                        tb[:, 0:GB, :], op=ALU.mult)
                nc.vector.tensor_tensor(s4[:, u, GB:QB, :], vfull[:, GB:QB, u, :],
                                        tb[:, GB:QB, :], op=ALU.mult)
                nc.vector.tensor_reduce(
                    cc[:, u * GRP + q * QB: u * GRP + (q + 1) * QB],
                    s4[:, u, :, :], axis=AX.X, op=ALU.add)
            # bf16 copy for stage-3 (gather + u)
            vbf = vbf_pool.tile([128, QB * N], BF16, tag="vbf")
            nc.scalar.copy(vbf[:], venat[:])
            vbf_g[(g, q)] = vbf

        def emit_stage2(g):
            """softmax + top-16 + bf16 selector build for group g."""
            cc = cc_g[g]
            cmp_ps = ps_tr.tile([128, 256], F32, tag="tr")
            nc.tensor.transpose(cmp_ps[:, 0:128], cc[:, 0:GRP], ident_f[:])
            nc.tensor.transpose(cmp_ps[:, 128:256], cc[:, GRP:2 * GRP], ident_f[:])

            dead_i = grp1.tile([GRP, N], I32, tag="deadi")
            nc.sync.dma_start(dead_i[:], dead_d.ap()[g * GRP:(g + 1) * GRP, :])
            dead_f = grp1.tile([GRP, N], F32, tag="deadf")
            nc.vector.tensor_copy(dead_f[:], dead_i[:])
            cm_sb = grp1.tile([GRP, N], F32, tag="cmsb")
            nc.vector.scalar_tensor_tensor(cm_sb[:], dead_f[:], NEG_MASK,
                                           cmp_ps[:, :], op0=ALU.mult, op1=ALU.add)

            mx_neg = small.tile([GRP, 1], F32, tag="mxneg")
            nc.vector.tensor_reduce(mx_neg[:], cm_sb[:], axis=AX.X, op=ALU.max,
                                    negate=True)
            score_un = grp1.tile([GRP, N], F32, tag="scoreun")
            ssum = small.tile([GRP, 1], F32, tag="ssum")
            nc.scalar.activation(score_un[:], cm_sb[:], ACTF.Exp,
                                 bias=mx_neg[:], scale=1.0, accum_out=ssum[:])
            rs = small.tile([GRP, 1], F32, tag="rs")
            nc.vector.reciprocal(rs[:], ssum[:])
            score_bf = grp1.tile([GRP, N], BF16, tag="scorebf")
            nc.vector.tensor_scalar_mul(score_bf[:], score_un[:], rs[:])

            # top-16 (two rounds of max8 + find_index8)
            mx8a = small.tile([GRP, 8], F32, tag="mx8a")
            nc.vector.max(mx8a[:], cm_sb[:])
            idx16 = small.tile([GRP, K16], U16, tag="idx16")
            nc.vector.max_index(idx16[:, 0:8], mx8a[:], cm_sb[:])
            cm2 = grp1.tile([GRP, N], F32, tag="cm2")
            nc.vector.match_replace(cm2[:], mx8a[:], cm_sb[:], NEG_REPL)
            mx8b = small.tile([GRP, 8], F32, tag="mx8b")
            nc.vector.max(mx8b[:], cm2[:])
            nc.vector.max_index(idx16[:, 8:16], mx8b[:], cm2[:])
            idx_bf = small.tile([GRP, K16], BF16, tag="idxbf")
            nc.vector.tensor_copy(idx_bf[:], idx16[:])
            # bounce idx rows to DRAM, reload flat on one partition
            idx_dram = dram_pool.tile([GRP, K16], BF16, tag="idxdram")
            nc.sync.dma_start(idx_dram[:], idx_bf[:])
            idx_pack = tpk_pool.tile([1, GRP * K16], BF16, tag="idxpack")
            nc.sync.dma_start(idx_pack[:], idx_dram[:].rearrange("b k -> (b k)"))
            # broadcast indices to all partitions: [128, (ba, j)]
            idx_ps_a = ps_tbs.tile([128, 1024], F32, tag="tbs")
            idx_ps_b = ps_tbs.tile([128, 1024], F32, tag="tbs")
            for half, ps in ((0, idx_ps_a), (1, idx_ps_b)):
                for qq in range(2):
                    lo = half * 1024 + qq * 512
                    nc.tensor.matmul(ps[:, qq * 512:(qq + 1) * 512],
                                     ones1_bf[:], idx_pack[:, lo:lo + 512],
                                     start=True, stop=True)
            idx_sb = tbs_pool.tile([128, GRP * K16], BF16, tag="idxsb")
            nc.scalar.copy(idx_sb[:, 0:1024], idx_ps_a[:])
            nc.scalar.copy(idx_sb[:, 1024:2048], idx_ps_b[:])
            # selectors: s[p, ba, j] = (idx[ba, j] == n(p)) ; col 17 = score
            s_a = grp1.tile([128, GRP * (K16 + 1)], BF16, tag="sa")
            s_b = grp1.tile([128, GRP * (K16 + 1)], BF16, tag="sb")
            s_a_v = s_a[:].rearrange("p (b j) -> p b j", j=K16 + 1)
            s_b_v = s_b[:].rearrange("p (b j) -> p b j", j=K16 + 1)
            idx_v = idx_sb[:].rearrange("p (b j) -> p b j", j=K16)
            nc.vector.tensor_scalar(s_a_v[:, :, 0:K16], idx_v, iota_p[:], None,
                                    op0=ALU.is_equal)
            nc.vector.tensor_scalar(s_b_v[:, :, 0:K16], idx_v, iota_p2[:], None,
                                    op0=ALU.is_equal)
            # score columns: transpose [ba, n] -> [n, ba] (bf16)
            st_ps = ps_trb.tile([128, 256], BF16, tag="trb")
            nc.tensor.transpose(st_ps[:, 0:128], score_bf[:, 0:128], ident_bf[:])
            nc.tensor.transpose(st_ps[:, 128:256], score_bf[:, 128:256],
                                ident_bf[:])
            nc.scalar.copy(s_a_v[:, :, K16], st_ps[:, 0:128])
            nc.scalar.copy(s_b_v[:, :, K16], st_ps[:, 128:256])
            sel_g[g] = (s_a, s_b)
            xq_g[g] = grp_pool.tile([128, NCH * 128], BF16, tag="xq", name="xq")

        def emit_stage3_chunk(g, c):
            """gather+u for ba in [c*CH, (c+1)*CH): sel-stationary bf16 MMs,
            4 ba packed in one psum tile via col tile_position, then one PE
            transpose back to [h, (ba-chunk cols)]."""
            s_a, s_b = sel_g[g]
            par = c % 2
            xps = xps_all[:][:, par * 128:(par + 1) * 128]
            for q4 in range(CH):
                ba = c * CH + q4
                vb = vbf_g[(g, ba // QB)]
                base = (ba % QB) * N
                lo, hi = ba * 17, (ba + 1) * 17
                xps_q = xps[32 * q4:32 * q4 + 17, :]
                nc.tensor.matmul(xps_q, s_a[:, lo:hi], vb[:, base:base + 128],
                                 start=True, stop=False,
                                 tile_position=(0, 32 * q4))
                nc.tensor.matmul(xps_q, s_b[:, lo:hi], vb[:, base + 128:base + 256],
                                 start=False, stop=True,
                                 tile_position=(0, 32 * q4))
            x_sb = xsb_pool.tile([128, 128], BF16, tag="xsb")
            nc.scalar.copy(x_sb[:], xps)
            xt_ps = xt_all[:][:, par * 128:(par + 1) * 128]
            nc.tensor.transpose(xt_ps, x_sb[:], ident_bf[:])
            nc.vector.tensor_copy(xq_g[g][:, c * 128:(c + 1) * 128], xt_ps)

        def emit_heads(g):
            """vC / vM heads for group g. xq col = c*128 + 32*q4 + j."""
            xq = xq_g[g]
            xq_v = xq[:].rearrange("p (c q w) -> p c q w", q=CH, w=32)
            vc_ps = ps_tr.tile([128, 256], F32, tag="tr")
            nc.tensor.matmul(vc_ps[:, 0:GRP], wf0T[:], vst_f[g][:],
                             start=True, stop=False)
            for j in range(1, K16 + 1):
                nc.tensor.matmul(vc_ps[:, 0:GRP],
                                 wf_bf[:, (j - 1) * H:j * H],
                                 xq_v[:, :, :, j - 1],
                                 start=False, stop=(j == K16))
            vc_sb = grp1.tile([128, GRP], F32, tag="vcsb")
            nc.scalar.activation(vc_sb[:], vc_ps[:, 0:GRP], ACTF.Relu,
                                 bias=bfwd[:], scale=1.0)
            vc_rows = grp1.tile([GRP, H], F32, tag="vcrows")
            pe_transpose(vc_rows[:], vc_sb[:])
            nc.sync.dma_start(vc_d.ap()[g * GRP:(g + 1) * GRP, :], vc_rows[:])

            vm_ps = ps_tr.tile([128, 256], F32, tag="tr")
            nc.tensor.matmul(vm_ps[:, 0:GRP], wm0T[:], vst_f[g][:],
                             start=True, stop=False)
            nc.tensor.matmul(vm_ps[:, 0:GRP], wmv_bf[:], xq_v[:, :, :, K16],
                             start=False, stop=True)
            vm_sb = grp1.tile([128, GRP], F32, tag="vmsb")
            nc.scalar.activation(vm_sb[:], vm_ps[:, 0:GRP], ACTF.Relu,
                                 bias=bmot[:], scale=1.0)
            vm_rows = grp1.tile([GRP, H], F32, tag="vmrows")
            pe_transpose(vm_rows[:], vm_sb[:])
            nc.sync.dma_start(vm_d.ap()[g * GRP:(g + 1) * GRP, :], vm_rows[:])

        # ---- software-pipelined emission ----
        for q in range(NQ):
            emit_stage1_qb(0, q)
        for g in range(NG):
            emit_stage2(g)
            for q in range(NQ):
                emit_stage3_chunk(g, 2 * q)
                emit_stage3_chunk(g, 2 * q + 1)
                if g + 1 < NG:
                    emit_stage1_qb(g + 1, q)
            emit_heads(g)
            for q in range(NQ):
                del vbf_g[(g, q)]


def _get_compiled(B_pc):
    key = B_pc
    if key not in _CACHE:
        nc = bacc.Bacc("TRN2", target_bir_lowering=False, debug=False,
                       num_devices=N_CORES)
        _build(nc, B_pc)
        nc.compile()
        _CACHE[key] = nc
    return _CACHE[key]


def kernel(vs, ve, ve_dead, Wq, Wk, Wv, W_mot, b_mot, W_fwd, b_fwd,
           trace=False, trace_kwargs=None):
    vs = np.asarray(vs, dtype=np.float32)
    ve = np.asarray(ve, dtype=np.float32)
    ve_dead = np.asarray(ve_dead, dtype=np.int32)
    Bq, Aq = vs.shape[0], vs.shape[1]
    assert (Bq, Aq) == (B, A), (Bq, Aq)
    B_pc = B // N_CORES
    NBA = B_pc * A

    nc = _get_compiled(B_pc)

    shared = {
        "wq": np.ascontiguousarray(Wq, dtype=np.float32),
        "wk": np.ascontiguousarray(Wk, dtype=np.float32),
        "wv": np.ascontiguousarray(Wv, dtype=np.float32),
        "wmot": np.ascontiguousarray(W_mot, dtype=np.float32),
        "bmot": np.ascontiguousarray(b_mot, dtype=np.float32).reshape(H, 1),
        "wfwd": np.ascontiguousarray(W_fwd, dtype=np.float32),
        "bfwd": np.ascontiguousarray(b_fwd, dtype=np.float32).reshape(H, 1),
    }
    in_maps = []
    for c in range(N_CORES):
        sl = slice(c * B_pc, (c + 1) * B_pc)
        in_maps.append({
            "ve": np.ascontiguousarray(ve[sl].reshape(NBA, N, H)),
            "vs": np.ascontiguousarray(vs[sl].reshape(NBA, H)),
            "dead": np.ascontiguousarray(ve_dead[sl].reshape(NBA, N)),
            **shared,
        })

    res = bass_utils.run_bass_kernel_spmd(
        nc, in_maps, core_ids=list(range(N_CORES)),
        trace=trace, **(trace_kwargs or {}))

    vc = np.empty((B, A, H), dtype=np.float32)
    vm = np.empty((B, A, H), dtype=np.float32)
    for c in range(N_CORES):
        sl = slice(c * B_pc, (c + 1) * B_pc)
        vc[sl] = res.results[c]["vc"].reshape(B_pc, A, H)
        vm[sl] = res.results[c]["vm"].reshape(B_pc, A, H)
    kernel.last_results = res
    return (vc, vm)


# revision 11
# speedup vs baseline: 2.1329x; 1.0324x over previous
"""Trainium2 Bass kernel for nn_Concentration_61229053772314.

kernel(**inputs) takes the FULL inputs (B=64), shards the batch dim across
8 NeuronCores (pure data parallel, weights replicated), runs a Bass/Tile
kernel via run_bass_kernel_spmd, and reassembles the full outputs.

v3 architecture (per core: NBA=256 (b,a) rows, 2 groups of GRP=128):
 - ve streamed once as f32 in [n mod 128, (b, u, h)] layout, 1MB DMAs;
   each tile is consumed by stage-1 (compat) + cast to a bf16 copy for
   stage-3, then freed (f32 venat is transient).
 - compat = sum_h ve*t: t broadcast via ONE K=2 PE matmul per 512 cols
   (hi/lo f32r planes stacked on 2 partitions -> exact f32 sum); the
   multiply+reduce is split between gpsimd (plain mult, u=0 half) and
   DVE tensor_tensor_reduce (fused mult+reduce, u=1 half).
 - softmax/top-16 on [128, 256] rows; one-hot selectors built on DVE in
   bf16 (indices <= 255 are exact in bf16).
 - stage-3 gather: selector is the 17-col STATIONARY (cheap LDWEIGHTS),
   ve_bf16 is the moving operand (1 cyc/row). 4 ba per PSUM tile via
   col tile_position; one PE transpose per chunk restores [h, (ba,j)].
   Gathered values / u are bf16 (tol 2e-2); compat/top-k stay f32.
 - heads: W_fwd blocks 1..16 and W_mot@WvT in bf16 against gathered x;
   the vs-terms stay exact f32.
"""
import math
import os
import sys

for _p in ("/opt/trn_rl_repo", "/root/.axon_site/_ro/trn_rl_repo"):
    if os.path.isdir(_p) and _p not in sys.path:
        sys.path.insert(0, _p)

import numpy as np
import concourse.tile as tile
from concourse import bacc, bass_utils, mybir

F32 = mybir.dt.float32
F32R = mybir.dt.float32r
BF16 = mybir.dt.bfloat16
I32 = mybir.dt.int32
U16 = mybir.dt.uint16
AX = mybir.AxisListType
ALU = mybir.AluOpType
ACTF = mybir.ActivationFunctionType

N_CORES = 8
B, A = 64, 32
N = 256    # entries per (b,a)
H = 128    # head dim
K16 = 16   # top-k
GRP = 128  # (b,a) pairs per processing group
QB = 8     # ba per DMA / pipeline step
NQ = GRP // QB  # steps per group
CH = 4     # ba per stage-3 psum chunk
NCH = GRP // CH

NEG_MASK = -1.0e30   # added to masked entries
NEG_REPL = -3.0e38   # match_replace fill (below any real/masked value)

_CACHE = {}


def _build(nc, B_pc):
    NBA = 32 * B_pc
    assert NBA % GRP == 0
    NG = NBA // GRP

    ve_d = nc.dram_tensor("ve", [NBA, N, H], F32, kind="ExternalInput")
    vs_d = nc.dram_tensor("vs", [NBA, H], F32, kind="ExternalInput")
    dead_d = nc.dram_tensor("dead", [NBA, N], I32, kind="ExternalInput")
    wq_d = nc.dram_tensor("wq", [H, H], F32, kind="ExternalInput")
    wk_d = nc.dram_tensor("wk", [H, H], F32, kind="ExternalInput")
    wv_d = nc.dram_tensor("wv", [H, H], F32, kind="ExternalInput")
    wmot_d = nc.dram_tensor("wmot", [H, 2 * H], F32, kind="ExternalInput")
    bmot_d = nc.dram_tensor("bmot", [H, 1], F32, kind="ExternalInput")
    wfwd_d = nc.dram_tensor("wfwd", [H, (K16 + 1) * H], F32, kind="ExternalInput")
    bfwd_d = nc.dram_tensor("bfwd", [H, 1], F32, kind="ExternalInput")
    vc_d = nc.dram_tensor("vc", [NBA, H], F32, kind="ExternalOutput")
    vm_d = nc.dram_tensor("vm", [NBA, H], F32, kind="ExternalOutput")

    with tile.TileContext(nc) as tc:
        _body(nc, tc, NBA, NG, ve_d, vs_d, dead_d, wq_d, wk_d, wv_d,
              wmot_d, bmot_d, wfwd_d, bfwd_d, vc_d, vm_d)


def _body(nc, tc, NBA, NG, ve_d, vs_d, dead_d, wq_d, wk_d, wv_d,
          wmot_d, bmot_d, wfwd_d, bfwd_d, vc_d, vm_d):
    from contextlib import ExitStack
    with ExitStack() as ctx:
        consts = ctx.enter_context(tc.tile_pool(name="consts", bufs=1))
        wres = ctx.enter_context(tc.tile_pool(name="wres", bufs=1))
        venat_pool = ctx.enter_context(tc.tile_pool(name="venat", bufs=3))
        vbf_pool = ctx.enter_context(tc.tile_pool(name="vbf", bufs=18))
        scr_pool = ctx.enter_context(tc.tile_pool(name="scr", bufs=2))
        tbs_pool = ctx.enter_context(tc.tile_pool(name="tbs", bufs=2))
        tpk_pool = ctx.enter_context(tc.tile_pool(name="tpk", bufs=2))
        tpre = ctx.enter_context(tc.tile_pool(name="tpre", bufs=1))
        vst_pool = ctx.enter_context(tc.tile_pool(name="vst", bufs=2))
        grp_pool = ctx.enter_context(tc.tile_pool(name="grp", bufs=2))
        grp1 = ctx.enter_context(tc.tile_pool(name="grp1", bufs=1))
        xsb_pool = ctx.enter_context(tc.tile_pool(name="xsb", bufs=3))
        small = ctx.enter_context(tc.tile_pool(name="small", bufs=3))
        dram_pool = ctx.enter_context(tc.tile_pool(name="dram", bufs=4, space="DRAM"))
        # PSUM budget, bank-granular (8 banks of 2KB/partition):
        #   ps_tbs [128,1024]f32 (2 banks) x2 bufs     = 4 banks
        #   ps_tr  [128,256]f32 x1                     = 1 bank
        #   ps_trb [128,256]bf16 x1                    = 1 bank
        #   ps_x   [128,256]f32 persistent (2 chunks)  = 1 bank
        #   ps_xt  [128,512]bf16 persistent (2 chunks) = 1 bank
        ps_tbs = ctx.enter_context(tc.tile_pool(name="ps_tbs", bufs=1, space="PSUM"))
        ps_scr = ctx.enter_context(tc.tile_pool(name="ps_scr", bufs=1, space="PSUM"))
        ps_tr = ctx.enter_context(tc.tile_pool(name="ps_tr", bufs=1, space="PSUM"))
        ps_trb = ctx.enter_context(tc.tile_pool(name="ps_trb", bufs=1, space="PSUM"))
        ps_x = ctx.enter_context(tc.tile_pool(name="ps_x", bufs=1, space="PSUM"))
        ps_xt = ctx.enter_context(tc.tile_pool(name="ps_xt", bufs=1, space="PSUM"))

        # ---- constants ----
        iota_n = consts.tile([128, 128], I32)
        nc.gpsimd.iota(iota_n[:], pattern=[[1, 128]], base=0, channel_multiplier=0)
        iota_p = consts.tile([128, 1], F32)
        nc.gpsimd.iota(iota_p[:], pattern=[[0, 1]], base=0, channel_multiplier=1,
                       allow_small_or_imprecise_dtypes=True)
        iota_p2 = consts.tile([128, 1], F32)  # p + 128
        nc.gpsimd.iota(iota_p2[:], pattern=[[0, 1]], base=128, channel_multiplier=1,
                       allow_small_or_imprecise_dtypes=True)
        iota_p_bf = consts.tile([128, 1], BF16)
        nc.vector.tensor_copy(iota_p_bf[:], iota_p[:])
        iota_p2_bf = consts.tile([128, 1], BF16)
        nc.vector.tensor_copy(iota_p2_bf[:], iota_p2[:])
        ident_f = consts.tile([128, 128], F32)
        nc.vector.tensor_scalar(ident_f[:], iota_n[:], iota_p[:], None,
                                op0=ALU.is_equal)
        ident_bf = consts.tile([128, 128], BF16)
        nc.vector.tensor_copy(ident_bf[:], ident_f[:])
        ones2_f = consts.tile([2, 128], F32)
        nc.gpsimd.memset(ones2_f[:], 1.0)
        ones2_r = consts.tile([2, 128], F32R)
        nc.scalar.copy(ones2_r[:], ones2_f[:])
        ones1_bf = consts.tile([1, 128], BF16)
        nc.gpsimd.memset(ones1_bf[:], 1.0)

        def pe_transpose(dst_sb, src_sb, eng=nc.scalar):
            """dst[f, p] = src[p, f] via PE; dst in SBUF (f32 path)."""
            p_in, f_in = src_sb.shape[0], src_sb.shape[1]
            ps = ps_tr.tile([128, 256], F32, tag="tr")
            out = ps[0:f_in, 0:p_in]
            nc.tensor.transpose(out, src_sb, ident_f[0:p_in, 0:p_in])
            eng.copy(dst_sb, out)

        # ---- weights (temp pool closed after preamble) ----
        with tc.tile_pool(name="wtmp", bufs=1) as wtmp:
            wq = wres.tile([H, H], F32)
            nc.sync.dma_start(wq[:], wq_d.ap())
            wk = wtmp.tile([H, H], F32)
            nc.sync.dma_start(wk[:], wk_d.ap())
            wv = wtmp.tile([H, H], F32)
            nc.sync.dma_start(wv[:], wv_d.ap())
            wmot = wtmp.tile([H, 2 * H], F32)
            nc.sync.dma_start(wmot[:], wmot_d.ap())
            wfwd = wtmp.tile([H, (K16 + 1) * H], F32)
            nc.sync.dma_start(wfwd[:], wfwd_d.ap())
            bmot = wres.tile([H, 1], F32)
            nc.sync.dma_start(bmot[:], bmot_d.ap())
            bfwd = wres.tile([H, 1], F32)
            nc.sync.dma_start(bfwd[:], bfwd_d.ap())

            wkT = wres.tile([H, H], F32)
            pe_transpose(wkT[:], wk[:])
            wvT = wtmp.tile([H, H], F32)
            pe_transpose(wvT[:], wv[:])
            wm0T = wres.tile([H, H], F32)
            pe_transpose(wm0T[:], wmot[:, 0:H])
            wm1T = wtmp.tile([H, H], F32)
            pe_transpose(wm1T[:], wmot[:, H:2 * H])

            # WmvT = (Wm1 @ Wv^T)^T, in bf16 (multiplies bf16 u)
            wmvT_f = wtmp.tile([H, H], F32)
            ps = ps_tr.tile([128, 256], F32, tag="tr")
            nc.tensor.matmul(ps[:, 0:128], wvT[:], wm1T[:])
            nc.scalar.copy(wmvT_f[:], ps[:, 0:128])
            wmv_bf = wres.tile([H, H], BF16)
            nc.scalar.copy(wmv_bf[:], wmvT_f[:])

            # W_fwd block transposes; block 0 stays f32 (vs term), blocks
            # 1..16 drain to bf16 (they multiply bf16 gathered rows)
            wf0T = wres.tile([H, H], F32)
            pe_transpose(wf0T[:], wfwd[:, 0:H])
            wf_bf = wres.tile([H, K16 * H], BF16)
            for j in range(1, K16 + 1):
                pe_transpose(wf_bf[:, (j - 1) * H:j * H],
                             wfwd[:, j * H:(j + 1) * H])

        # ---- per-group t precompute ----
        vst_f, t_dram_g = [], []
        for g in range(NG):
            vs_rows = tpre.tile([GRP, H], F32, tag="vsrows")
            nc.sync.dma_start(vs_rows[:], vs_d.ap()[g * GRP:(g + 1) * GRP, :])
            vstf = vst_pool.tile([H, GRP], F32, tag="vstf")
            pe_transpose(vstf[:], vs_rows[:])
            qt = tpre.tile([H, GRP], F32, tag="qt")
            ps = ps_tr.tile([128, 256], F32, tag="tr")
            nc.tensor.matmul(ps[:, 0:GRP], wq[:], vstf[:])
            nc.scalar.copy(qt[:], ps[:, 0:GRP])
            tsb = tpre.tile([H, GRP], F32, tag="tsb")
            ps = ps_tr.tile([128, 256], F32, tag="tr")
            nc.tensor.matmul(ps[:, 0:GRP], wkT[:], qt[:])
            nc.scalar.mul(tsb[:], ps[:, 0:GRP], 1.0 / math.sqrt(H))
            # t rows [ba, h]: split into f32r hi + lo residual (exact in f32)
            trows_f = tpre.tile([GRP, H], F32, tag="trowsf")
            pe_transpose(trows_f[:], tsb[:])
            trows_r = tpre.tile([GRP, H], F32R, tag="trowsr")
            nc.scalar.copy(trows_r[:], trows_f[:])
            tlo_r = tpre.tile([GRP, H], F32R, tag="tlor")
            nc.vector.tensor_tensor(tlo_r[:], trows_f[:], trows_r[:].bitcast(F32),
                                    op=ALU.subtract)
            # t_dram[u, ba, h]: u=0 hi plane, u=1 lo plane
            t_dram = dram_pool.tile([2, GRP, H], F32R, tag="tdram")
            nc.sync.dma_start(t_dram[:][0], trows_r[:])
            nc.sync.dma_start(t_dram[:][1], tlo_r[:])
            vst_f.append(vstf)
            t_dram_g.append(t_dram)

        # ---- per-group state ----
        xps_all = ps_x.tile([128, 256], F32, tag="x")
        xt_all = ps_xt.tile([128, 256], BF16, tag="xt")
        vbf_g = {}       # (g, t8) -> bf16 venat tile [128, QB*N]
        cc_g = {}        # g -> cc tile [128, 2*GRP] ([n-half, (u, ba)])
        sel_g = {}       # g -> (s_a, s_b) bf16 [128, GRP*17]
        xq_g = {}        # g -> gathered tile [128, NCH*128] bf16

        def emit_stage1_qb(g, q):
            """load QB ba's of ve, broadcast t, multiply+reduce, cast bf16."""
            ib = g * GRP + q * QB
            if q == 0:
                cc_g[g] = grp_pool.tile([128, 2 * GRP], F32, tag="cc", name="cc")
            cc = cc_g[g]
            venat = venat_pool.tile([128, QB * N], F32, tag="venat")
            src = ve_d.ap()[ib:ib + QB].rearrange("b (u n) h -> n b u h", u=2)
            nc.sync.dma_start(
                venat[:].rearrange("p (b u h) -> p b u h", b=QB, u=2), src)
            # t pack for these QB rows: [2, QB*H] f32r (hi plane, lo plane)
            tpk = tpk_pool.tile([2, QB * H], F32R, tag="tpk")
            nc.sync.dma_start(
                tpk[:].rearrange("p (b h) -> p b h", b=QB),
                t_dram_g[g][:][:, q * QB:(q + 1) * QB, :])
            # broadcast t across partitions: K=2 matmul sums hi+lo exactly
            tbs_ps = ps_tbs.tile([128, QB * H], F32, tag="tbs")
            nc.tensor.matmul(tbs_ps[:, 0:512], ones2_r[:], tpk[:, 0:512],
                             start=True, stop=True)
            nc.tensor.matmul(tbs_ps[:, 512:1024], ones2_r[:], tpk[:, 512:1024],
                             start=True, stop=True)
            tbs = tbs_pool.tile([128, QB * H], F32, tag="tbs")
            nc.scalar.copy(tbs[:], tbs_ps[:])
            vfull = venat[:].rearrange("p (b u h) -> p b u h", b=QB, u=2)
            tb = tbs[:].rearrange("p (b h) -> p b h", b=QB)
            tbp = tbs_ps[:].rearrange("p (b h) -> p b h", b=QB)
            # gpsimd: u0 all b + u1 b[0:GB1] (SBUF tbs); DVE: u1 b[GB1:]
            # reading tbs from PSUM and writing scratch to PSUM (keeps the
            # shared DVE/gpsimd SBUF port free for ve reads).
            GB1 = 2
            scr = scr_pool.tile([128, (QB + GB1) * H], F32, tag="scr")
            s0 = scr[:].rearrange("p (b h) -> p b h", b=QB + GB1)
            scrp = ps_scr.tile([128, (QB - GB1) * H], F32, tag="scrp")
            sp = scrp[:].rearrange("p (b h) -> p b h", b=QB - GB1)
            nc.gpsimd.tensor_tensor(s0[:, 0:QB, :], vfull[:, :, 0, :], tb,
                                    op=ALU.mult)
            nc.gpsimd.tensor_tensor(s0[:, QB:QB + GB1, :], vfull[:, 0:GB1, 1, :],
                                    tb[:, 0:GB1, :], op=ALU.mult)
            nc.vector.tensor_tensor(sp[:, :, :], vfull[:, GB1:QB, 1, :],
                                    tbp[:, GB1:QB, :], op=ALU.mult)
            nc.vector.tensor_reduce(cc[:, q * QB:(q + 1) * QB], s0[:, 0:QB, :],
                                    axis=AX.X, op=ALU.add)
            nc.vector.tensor_reduce(
                cc[:, GRP + q * QB: GRP + q * QB + GB1],
                s0[:, QB:QB + GB1, :], axis=AX.X, op=ALU.add)
            nc.vector.tensor_reduce(
                cc[:, GRP + q * QB + GB1: GRP + (q + 1) * QB],
                sp[:, :, :], axis=AX.X, op=ALU.add)
            # bf16 copy for stage-3 (gather + u)
            vbf = vbf_pool.tile([128, QB * N], BF16, tag="vbf")
            nc.scalar.copy(vbf[:], venat[:])
            vbf_g[(g, q)] = vbf

        def emit_stage2(g):
            """softmax + top-16 + bf16 selector build for group g."""
            cc = cc_g[g]
            cmp_ps = ps_tr.tile([128, 256], F32, tag="tr")
            nc.tensor.transpose(cmp_ps[:, 0:128], cc[:, 0:GRP], ident_f[:])
            nc.tensor.transpose(cmp_ps[:, 128:256], cc[:, GRP:2 * GRP], ident_f[:])

            dead_i = grp1.tile([GRP, N], I32, tag="deadi")
            nc.sync.dma_start(dead_i[:], dead_d.ap()[g * GRP:(g + 1) * GRP, :])
            dead_f = grp1.tile([GRP, N], F32, tag="deadf")
            nc.vector.tensor_copy(dead_f[:], dead_i[:])
            cm_sb = grp1.tile([GRP, N], F32, tag="cmsb")
            nc.vector.scalar_tensor_tensor(cm_sb[:], dead_f[:], NEG_MASK,
                                           cmp_ps[:, :], op0=ALU.mult, op1=ALU.add)

            mx_neg = small.tile([GRP, 1], F32, tag="mxneg")
            nc.vector.tensor_reduce(mx_neg[:], cm_sb[:], axis=AX.X, op=ALU.max,
                                    negate=True)
            score_un = grp1.tile([GRP, N], F32, tag="scoreun")
            ssum = small.tile([GRP, 1], F32, tag="ssum")
            nc.scalar.activation(score_un[:], cm_sb[:], ACTF.Exp,
                                 bias=mx_neg[:], scale=1.0, accum_out=ssum[:])
            rs = small.tile([GRP, 1], F32, tag="rs")
            nc.vector.reciprocal(rs[:], ssum[:])
            score_bf = grp1.tile([GRP, N], BF16, tag="scorebf")
            nc.vector.tensor_scalar_mul(score_bf[:], score_un[:], rs[:])

            # top-16 (two rounds of max8 + find_index8)
            mx8a = small.tile([GRP, 8], F32, tag="mx8a")
            nc.vector.max(mx8a[:], cm_sb[:])
            idx16 = small.tile([GRP, K16], U16, tag="idx16")
            nc.vector.max_index(idx16[:, 0:8], mx8a[:], cm_sb[:])
            cm2 = grp1.tile([GRP, N], F32, tag="cm2")
            nc.vector.match_replace(cm2[:], mx8a[:], cm_sb[:], NEG_REPL)
            mx8b = small.tile([GRP, 8], F32, tag="mx8b")
            nc.vector.max(mx8b[:], cm2[:])
            nc.vector.max_index(idx16[:, 8:16], mx8b[:], cm2[:])
            idx_bf = small.tile([GRP, K16], BF16, tag="idxbf")
            nc.vector.tensor_copy(idx_bf[:], idx16[:])
            # flatten idx rows onto one partition via SBUF->SBUF DMA
            idx_pack = tpk_pool.tile([1, GRP * K16], BF16, tag="idxpack")
            nc.sync.dma_start(
                idx_pack[:].rearrange("p (b k) -> p b k", k=K16), idx_bf[:])
            # broadcast indices to all partitions: [128, (ba, j)]
            idx_ps_a = ps_tbs.tile([128, 1024], F32, tag="tbs")
            idx_ps_b = ps_tbs.tile([128, 1024], F32, tag="tbs")
            for half, ps in ((0, idx_ps_a), (1, idx_ps_b)):
                for qq in range(2):
                    lo = half * 1024 + qq * 512
                    nc.tensor.matmul(ps[:, qq * 512:(qq + 1) * 512],
                                     ones1_bf[:], idx_pack[:, lo:lo + 512],
                                     start=True, stop=True)
            idx_sb = tbs_pool.tile([128, GRP * K16], BF16, tag="idxsb")
            nc.scalar.copy(idx_sb[:, 0:1024], idx_ps_a[:])
            nc.scalar.copy(idx_sb[:, 1024:2048], idx_ps_b[:])
            # selectors: s[p, ba, j] = (idx[ba, j] == n(p)) ; col 17 = score
            s_a = grp1.tile([128, GRP * (K16 + 1)], BF16, tag="sa")
            s_b = grp1.tile([128, GRP * (K16 + 1)], BF16, tag="sb")
            s_a_v = s_a[:].rearrange("p (b j) -> p b j", j=K16 + 1)
            s_b_v = s_b[:].rearrange("p (b j) -> p b j", j=K16 + 1)
            idx_v = idx_sb[:].rearrange("p (b j) -> p b j", j=K16)
            nc.vector.tensor_scalar(s_a_v[:, :, 0:K16], idx_v, iota_p[:], None,
                                    op0=ALU.is_equal)
            nc.vector.tensor_scalar(s_b_v[:, :, 0:K16], idx_v, iota_p2[:], None,
                                    op0=ALU.is_equal)
            # score columns: transpose [ba, n] -> [n, ba] (bf16)
            st_ps = ps_trb.tile([128, 256], BF16, tag="trb")
            nc.tensor.transpose(st_ps[:, 0:128], score_bf[:, 0:128], ident_bf[:])
            nc.tensor.transpose(st_ps[:, 128:256], score_bf[:, 128:256],
                                ident_bf[:])
            nc.scalar.copy(s_a_v[:, :, K16], st_ps[:, 0:128])
            nc.scalar.copy(s_b_v[:, :, K16], st_ps[:, 128:256])
            sel_g[g] = (s_a, s_b)
            xq_g[g] = grp_pool.tile([128, NCH * 128], BF16, tag="xq", name="xq")

        def emit_stage3_chunk(g, c):
            """gather+u for ba in [c*CH, (c+1)*CH): sel-stationary bf16 MMs,
            4 ba packed in one psum tile via col tile_position, then one PE
            transpose back to [h, (ba-chunk cols)]."""
            s_a, s_b = sel_g[g]
            par = c % 2
            xps = xps_all[:][:, par * 128:(par + 1) * 128]
            for phase in range(2):
                for q4 in range(CH):
                    ba = c * CH + q4
                    vb = vbf_g[(g, ba // QB)]
                    base = (ba % QB) * N
                    lo, hi = ba * 17, (ba + 1) * 17
                    xps_q = xps[32 * q4:32 * q4 + 17, :]
                    if phase == 0:
                        nc.tensor.matmul(xps_q, s_a[:, lo:hi],
                                         vb[:, base:base + 128],
                                         start=True, stop=False,
                                         tile_position=(0, 32 * q4))
                    else:
                        nc.tensor.matmul(xps_q, s_b[:, lo:hi],
                                         vb[:, base + 128:base + 256],
                                         start=False, stop=True,
                                         tile_position=(0, 32 * q4))
            x_sb = xsb_pool.tile([128, 128], BF16, tag="xsb")
            nc.scalar.copy(x_sb[:], xps)
            xt_ps = xt_all[:][:, par * 128:(par + 1) * 128]
            nc.tensor.transpose(xt_ps, x_sb[:], ident_bf[:])
            nc.vector.tensor_copy(xq_g[g][:, c * 128:(c + 1) * 128], xt_ps)

        def emit_heads(g):
            """vC / vM heads for group g. xq col = c*128 + 32*q4 + j."""
            xq = xq_g[g]
            xq_v = xq[:].rearrange("p (c q w) -> p c q w", q=CH, w=32)
            vc_ps = ps_tr.tile([128, 256], F32, tag="tr")
            nc.tensor.matmul(vc_ps[:, 0:GRP], wf0T[:], vst_f[g][:],
                             start=True, stop=False)
            for j in range(1, K16 + 1):
                nc.tensor.matmul(vc_ps[:, 0:GRP],
                                 wf_bf[:, (j - 1) * H:j * H],
                                 xq_v[:, :, :, j - 1],
                                 start=False, stop=(j == K16))
            vc_sb = grp1.tile([128, GRP], F32, tag="vcsb")
            nc.scalar.activation(vc_sb[:], vc_ps[:, 0:GRP], ACTF.Relu,
                                 bias=bfwd[:], scale=1.0)
            vc_rows = grp1.tile([GRP, H], F32, tag="vcrows")
            pe_transpose(vc_rows[:], vc_sb[:])
            nc.sync.dma_start(vc_d.ap()[g * GRP:(g + 1) * GRP, :], vc_rows[:])

            vm_ps = ps_tr.tile([128, 256], F32, tag="tr")
            nc.tensor.matmul(vm_ps[:, 0:GRP], wm0T[:], vst_f[g][:],
                             start=True, stop=False)
            nc.tensor.matmul(vm_ps[:, 0:GRP], wmv_bf[:], xq_v[:, :, :, K16],
                             start=False, stop=True)
            vm_sb = grp1.tile([128, GRP], F32, tag="vmsb")
            nc.scalar.activation(vm_sb[:], vm_ps[:, 0:GRP], ACTF.Relu,
                                 bias=bmot[:], scale=1.0)
            vm_rows = grp1.tile([GRP, H], F32, tag="vmrows")
            pe_transpose(vm_rows[:], vm_sb[:])
            nc.sync.dma_start(vm_d.ap()[g * GRP:(g + 1) * GRP, :], vm_rows[:])

        # ---- software-pipelined emission ----
        for q in range(NQ):
            emit_stage1_qb(0, q)
        for g in range(NG):
            emit_stage2(g)
            for q in range(NQ):
                emit_stage3_chunk(g, 2 * q)
                emit_stage3_chunk(g, 2 * q + 1)
                if g + 1 < NG:
                    emit_stage1_qb(g + 1, q)
            emit_heads(g)
            for q in range(NQ):
                del vbf_g[(g, q)]


def _get_compiled(B_pc):
    key = B_pc
    if key not in _CACHE:
        nc = bacc.Bacc("TRN2", target_bir_lowering=False, debug=False,
                       num_devices=N_CORES)
        _build(nc, B_pc)
        nc.compile()
        _CACHE[key] = nc
    return _CACHE[key]


def kernel(vs, ve, ve_dead, Wq, Wk, Wv, W_mot, b_mot, W_fwd, b_fwd,
           trace=False, trace_kwargs=None):
    vs = np.asarray(vs, dtype=np.float32)
    ve = np.asarray(ve, dtype=np.float32)
    ve_dead = np.asarray(ve_dead, dtype=np.int32)
    Bq, Aq = vs.shape[0], vs.shape[1]
    assert (Bq, Aq) == (B, A), (Bq, Aq)
    B_pc = B // N_CORES
    NBA = B_pc * A

    nc = _get_compiled(B_pc)

    shared = {
        "wq": np.ascontiguousarray(Wq, dtype=np.float32),
        "wk": np.ascontiguousarray(Wk, dtype=np.float32),
        "wv": np.ascontiguousarray(Wv, dtype=np.float32),
        "wmot": np.ascontiguousarray(W_mot, dtype=np.float32),
        "bmot": np.ascontiguousarray(b_mot, dtype=np.float32).reshape(H, 1),
        "wfwd": np.ascontiguousarray(W_fwd, dtype=np.float32),
        "bfwd": np.ascontiguousarray(b_fwd, dtype=np.float32).reshape(H, 1),
    }
    in_maps = []
    for c in range(N_CORES):
        sl = slice(c * B_pc, (c + 1) * B_pc)
        in_maps.append({
            "ve": np.ascontiguousarray(ve[sl].reshape(NBA, N, H)),
            "vs": np.ascontiguousarray(vs[sl].reshape(NBA, H)),
            "dead": np.ascontiguousarray(ve_dead[sl].reshape(NBA, N)),
            **shared,
        })

    res = bass_utils.run_bass_kernel_spmd(
        nc, in_maps, core_ids=list(range(N_CORES)),
        trace=trace, **(trace_kwargs or {}))

    vc = np.empty((B, A, H), dtype=np.float32)
    vm = np.empty((B, A, H), dtype=np.float32)
    for c in range(N_CORES):
        sl = slice(c * B_pc, (c + 1) * B_pc)
        vc[sl] = res.results[c]["vc"].reshape(B_pc, A, H)
        vm[sl] = res.results[c]["vm"].reshape(B_pc, A, H)
    kernel.last_results = res
    return (vc, vm)
